# revision 50
# baseline (speedup 1.0000x reference)
"""Trainium2 Bass kernel for GQA attention (B=2, L=2048, D=2048, H=16, KV=8, HD=128).

Sharding: tensor-parallel over heads across 8 cores (2 Q heads + 1 KV head per
core), flash-style attention per core, then two AllToAlls (one per local head)
to redistribute from head-sharding to token-sharding before the output
projection (each core computes 512 full output rows; host concatenates).

v2 restructure vs baseline:
- exp runs at N=1024 over a 2-bank PSUM score tile (two query tiles of the
  same head share one ACT op) halving ACT per-op overhead.
- V transpose via the DMA xbar (dma_start_transpose) instead of PE.
- QKV projection and attention are software-pipelined at chunk granularity
  (attention on batch b starts as soon as the proj tiles it needs exist).
- Startup DMAs split across queues (w on scalar, x on sync, LUT on gpsimd)
  with per-chunk weight loads so the first matmul issues ~2us in.
- wo is host-relayout so every weight DMA is fully contiguous.
"""
import math
import numpy as np

B, L, D = 2, 2048, 2048
H, KV, HD = 16, 8, 128
NCORES = 8
T = B * L            # 4096 tokens, b-major
TPC = T // NCORES    # 512 tokens per core after A2A
HPC = H // NCORES    # 2 local query heads
EPS = 1e-5
ROPE_BASE = 10000.0
SCALE = HD ** -0.5

TT = 512             # token tile (free dim)
NTT = L // TT        # 4 token tiles per batch
NDC = D // 128       # 16 contraction chunks
NFC = 4              # output col chunks of 128 in qkv proj (2 q heads + k + v)
NK = L // 128        # 16 key chunks per batch

_CACHE = {}
DEBUG = False


def _rope_tables():
    """cos/sin LUTs [64, L] computed exactly like the jax reference (f32, cpu)."""
    import jax
    import jax.numpy as jnp

    cpu = jax.devices("cpu")[0]
    with jax.default_device(cpu):
        base = ROPE_BASE * 1.0 ** (HD / (HD - 2))
        freqs = base ** (jnp.arange(0, HD, 2, dtype=jnp.float32) / HD)   # [64]
        pos = jnp.arange(L, dtype=jnp.float32)                           # [L]
        angles = pos[:, None] * freqs[None, :]                           # [L, 64]
        cos = np.asarray(jnp.cos(angles), dtype=np.float32).T.copy()     # [64, L]
        sin = np.asarray(jnp.sin(angles), dtype=np.float32).T.copy()
    return cos, sin


def _build_nc():
    import concourse.bass as bass
    import concourse.tile as tile
    import concourse.mybir as mybir
    import concourse.bass_isa as bass_isa
    from concourse import bacc
    from contextlib import ExitStack
    from itertools import chain

    f32 = mybir.dt.float32
    f16 = mybir.dt.float16
    Exp = mybir.ActivationFunctionType.Exp
    Ln = mybir.ActivationFunctionType.Ln
    mult = mybir.AluOpType.mult
    add = mybir.AluOpType.add
    sub = mybir.AluOpType.subtract

    from concourse import bacc as _bacc_mod

    if not getattr(_bacc_mod, "_act_table_patch", False):
        _orig_get = _bacc_mod.get_activation_tables

        def _patched_get(arch):
            t = _orig_get(arch)
            exp = mybir.ActivationFunctionType.Exp
            ln = mybir.ActivationFunctionType.Ln
            for name, funcs in t.items():
                if name != "natural_log_exp_and_others":
                    funcs.discard(exp)
                    funcs.discard(ln)
            return t

        _bacc_mod.get_activation_tables = _patched_get
        _bacc_mod._act_table_patch = True

    nc = bacc.Bacc(num_devices=NCORES)

    # host-relayout inputs (all DMAs contiguous)
    xT6 = nc.dram_tensor("xT6", [B * NTT, 128, NDC, TT], f16, kind="ExternalInput")
    wqkv = nc.dram_tensor("wqkv", [NDC, 128, 512], f16, kind="ExternalInput")
    woR = nc.dram_tensor("woR", [NDC, 2, 128, 8, 128], f16, kind="ExternalInput")
    # full-height rope LUTs: lcosF = [cos; cos], lsinW = [-sin; sin] so
    # rope is 3 full-width DVE ops (t_a + t_b with a partition-swapped rsrc)
    lcosF = nc.dram_tensor("lcosF", [128, L], f16, kind="ExternalInput")
    lsinW = nc.dram_tensor("lsinW", [128, L], f16, kind="ExternalInput")
    qn = nc.dram_tensor("qn", [HD, 1], f32, kind="ExternalInput")
    kn = nc.dram_tensor("kn", [HD, 1], f32, kind="ExternalInput")
    yT = nc.dram_tensor("yT", [D, TPC], f32, kind="ExternalOutput")

    cc_in = [nc.dram_tensor(f"cc_in{h}", [NCORES, HD, TPC], f16) for h in range(HPC)]
    cc_out = [nc.dram_tensor(f"cc_out{h}", [NCORES, HD, TPC], f16) for h in range(HPC)]

    if DEBUG:
        dbg_q = nc.dram_tensor("dbg_q", [128, HPC, T], f16, kind="ExternalOutput")
        dbg_k = nc.dram_tensor("dbg_k", [128, T], f16, kind="ExternalOutput")
        dbg_v = nc.dram_tensor("dbg_v", [128, T // 128, HD], f16, kind="ExternalOutput")
        dbg_sacc = nc.dram_tensor("dbg_sacc", [128, 2 * TT], f16, kind="ExternalOutput")
        dbg_ag = nc.dram_tensor("dbg_ag", [128, NDC, TPC], f16, kind="ExternalOutput")

    with tile.TileContext(nc) as tc, ExitStack() as ctx, nc.allow_low_precision(
        reason="f16 storage; all matmul accumulation is fp32 PSUM"
    ):
        consts = ctx.enter_context(tc.tile_pool(name="consts", bufs=1))
        xtp = ctx.enter_context(tc.tile_pool(name="xtp", bufs=2))
        qkvp = ctx.enter_context(tc.tile_pool(name="qkvp", bufs=1))
        ropep = ctx.enter_context(tc.tile_pool(name="ropep", bufs=2))
        halfp = ctx.enter_context(tc.tile_pool(name="halfp", bufs=4))
        statp = ctx.enter_context(tc.tile_pool(name="statp", bufs=2))
        expp = ctx.enter_context(tc.tile_pool(name="expp", bufs=4))
        saccp = ctx.enter_context(tc.tile_pool(name="saccp", bufs=2))
        attp = ctx.enter_context(tc.tile_pool(name="attp", bufs=2))
        wop = ctx.enter_context(tc.tile_pool(name="wop", bufs=16))
        wop2 = ctx.enter_context(tc.tile_pool(name="wop2", bufs=8))
        yp = ctx.enter_context(tc.tile_pool(name="yp", bufs=2))
        yep = ctx.enter_context(tc.tile_pool(name="yep", bufs=16))

        pacc = ctx.enter_context(tc.tile_pool(name="pacc", bufs=2, space="PSUM"))
        pop = ctx.enter_context(tc.tile_pool(name="pop", bufs=2, space="PSUM"))
        pstream = ctx.enter_context(tc.tile_pool(name="pstream", bufs=2, space="PSUM"))

        # ---- constants ----
        # (rope LUT loads are emitted mid-schedule, behind the first x tiles,
        # to keep the startup path clear)
        eps_t = consts.tile([1, 1], f32)
        nc.vector.memset(eps_t, EPS)
        ones_f = consts.tile([128, 1], f32)
        nc.vector.memset(ones_f, 1.0)
        ones = consts.tile([128, 1], f16)
        nc.vector.tensor_copy(out=ones, in_=ones_f)
        ones_k1_f = consts.tile([1, 128], f32)
        nc.vector.memset(ones_k1_f, 1.0)
        ones_k1 = consts.tile([1, 128], f16)
        nc.vector.tensor_copy(out=ones_k1, in_=ones_k1_f)
        cos_sb = consts.tile([128, L], f16)
        sin_sb = consts.tile([128, L], f16)
        qn_sb = consts.tile([HD, 1], f32)
        kn_sb = consts.tile([HD, 1], f32)

        def load_luts():
            nc.scalar.dma_start(out=cos_sb, in_=lcosF[:, :])
            nc.scalar.dma_start(out=sin_sb, in_=lsinW[:, :])
            nc.scalar.dma_start(out=qn_sb, in_=qn[:, :])
            nc.scalar.dma_start(out=kn_sb, in_=kn[:, :])

        # ---- persistent activations ----
        qh_t = [
            [
                qkvp.tile([128, L], f16, tag=f"ag{h}", bufs=2, name=f"qh{h}{b}")
                for b in range(B)
            ]
            for h in range(HPC)
        ]
        kh_t = [
            qkvp.tile([128, L], f16, tag=f"kh{b}", name=f"kh{b}") for b in range(B)
        ]
        v_t = [
            qkvp.tile([128, L // 128, HD], f16, tag=f"v{b}", name=f"v{b}")
            for b in range(B)
        ]

        # ---- qkv weights: per-dc chunk loads on the scalar queue ----
        w_sb = consts.tile([128, NDC, 512], f16)
        for dc in range(4):
            nc.scalar.dma_start(out=w_sb[:, dc, :], in_=wqkv.ap()[dc, :, :])
        load_luts()
        for dc in range(4, NDC):
            nc.scalar.dma_start(out=w_sb[:, dc, :], in_=wqkv.ap()[dc, :, :])

        deferred_tr = []

        def flush_transposes():
            for fn in deferred_tr:
                fn()
            deferred_tr.clear()

        def gen_proj(b, tt, nsplit=1):
            """QKV projection + RoPE + RMSNorm for one 512-token tile.

            Yields once per 8 emitted matmuls so attention units can
            interleave on the PE queue. The whole x tile arrives as ONE
            contiguous 2MB DMA (host relayout); the first tile splits it
            so the first matmul isn't gated on the full transfer."""
            pos0 = tt * TT
            j = b * NTT + tt
            flush_transposes()  # previous tile's v transposes (data ready)
            xt2 = xtp.tile([128, NDC, TT], f16, tag="xt")
            step = NDC // nsplit
            for s in range(nsplit):
                nc.sync.dma_start(
                    out=xt2[:, s * step:(s + 1) * step, :],
                    in_=xT6.ap()[j, :, s * step:(s + 1) * step, :],
                )
            for fc in range(NFC):
                pp = pacc.tile([128, TT], f32, tag="pacc", name=f"pp{fc}")
                for dc in range(NDC):
                    nc.tensor.matmul(
                        pp,
                        w_sb[:, dc, fc * 128:(fc + 1) * 128],
                        xt2[:, dc, :],
                        start=(dc == 0),
                        stop=(dc == NDC - 1),
                    )
                    if dc == 7:
                        yield
                if fc < 3:
                    # rope as 3 full-width DVE ops:
                    #   roped = rsrc*[cos;cos] + [x2;x1]*[-sin;sin]
                    # where [x2;x1] is a DMA partition-swap of rsrc.
                    rsrc = ropep.tile([128, TT], f16, tag="rsrc")
                    nc.vector.tensor_copy(out=rsrc, in_=pp)
                    rsw = ropep.tile([128, TT], f16, tag="rsw")
                    nc.sync.dma_start(out=rsw[0:64, :], in_=rsrc[64:128, :])
                    nc.sync.dma_start(out=rsw[64:128, :], in_=rsrc[0:64, :])
                    ta = halfp.tile([128, TT], f16, tag="half")
                    tb = halfp.tile([128, TT], f16, tag="half")
                    roped = ropep.tile([128, TT], f16, tag="roped")
                    nc.vector.tensor_tensor(
                        out=ta, in0=rsrc, in1=cos_sb[:, pos0:pos0 + TT], op=mult
                    )
                    nc.vector.tensor_tensor(
                        out=tb, in0=rsw, in1=sin_sb[:, pos0:pos0 + TT], op=mult
                    )
                    nc.vector.tensor_tensor(out=roped, in0=ta, in1=tb, op=add)
                    sq = ropep.tile([128, TT], f16, tag="sq")
                    nc.vector.tensor_tensor(out=sq, in0=roped, in1=roped, op=mult)
                    # sum of squares over HD (partition) via ones-matmul
                    pss = pstream.tile([1, TT], f32, tag="ps", name="pss")
                    nc.tensor.matmul(pss, ones, sq, start=True, stop=True)
                    # rstd = exp(-0.5*ln(ss/HD + eps)); Ln/Exp share one table
                    lnt = statp.tile([1, TT], f32, tag="stat")
                    nc.scalar.activation(
                        out=lnt, in_=pss, func=Ln, bias=eps_t, scale=1.0 / HD
                    )
                    srd = statp.tile([1, TT], f16, tag="stat")
                    nc.scalar.activation(out=srd, in_=lnt, func=Exp, scale=-0.5)
                    # broadcast rstd over partitions via K=1 matmul
                    pb = pstream.tile([128, TT], f32, tag="ps", name="pbp")
                    nc.tensor.matmul(pb, ones_k1, srd, start=True, stop=True)
                    w_head = qn_sb if fc < 2 else kn_sb
                    if fc < 2:
                        dst = qh_t[fc][b][:, pos0:pos0 + TT]
                    else:
                        dst = kh_t[b][:, pos0:pos0 + TT]
                    nc.vector.scalar_tensor_tensor(
                        out=dst, in0=roped, scalar=w_head, in1=pb,
                        op0=mult, op1=mult,
                    )
                else:
                    # v: copy out; DMA-xbar transposes are deferred so they
                    # never park the sync queue waiting on this cast
                    vt = ropep.tile([128, TT], f16, tag="vtt", bufs=2)
                    nc.vector.tensor_copy(out=vt, in_=pp)

                    def mk(vt=vt, b=b, tt=tt):
                        for i in range(TT // 128):
                            nc.sync.dma_start_transpose(
                                out=v_t[b][:, tt * 4 + i, :],
                                in_=vt[:, i * 128:(i + 1) * 128],
                            )

                    deferred_tr.append(mk)

        def gen_att(hc, b, tq0):
            """Attention superjob: one head, two query tiles (tq0, tq0+1).

            Scores for both query tiles of key-chunk tk land in one 2-bank
            PSUM tile -> one N=1024 exp. pv matmuls trail by one chunk so
            the PE never waits on the ACT engine. Softmax denominator via
            PE ones-matmuls (latency-free next to the score stream). Yields
            once per key chunk."""
            tq1 = tq0 + 1
            qs0 = qh_t[hc][b][:, tq0 * TT:(tq0 + 1) * TT]
            qs1 = qh_t[hc][b][:, tq1 * TT:(tq1 + 1) * TT]
            po0 = pop.tile([128, TT], f32, tag="po", name=f"po{hc}{b}{tq0}a")
            po1 = pop.tile([128, TT], f32, tag="po", name=f"po{hc}{b}{tq0}b")
            sacc = saccp.tile([128, 2 * TT], f16, tag="sacc")
            ets = [None] * NK
            for tk in range(NK):
                ps = pstream.tile([128, 2 * TT], f32, tag="ps")
                kc = kh_t[b][:, tk * 128:(tk + 1) * 128]
                nc.tensor.matmul(ps[:, 0:TT], kc, qs0, start=True, stop=True)
                nc.tensor.matmul(ps[:, TT:2 * TT], kc, qs1, start=True, stop=True)
                et = expp.tile([128, 2 * TT], f16, tag="et")
                nc.scalar.activation(out=et, in_=ps, func=Exp, scale=SCALE)
                ets[tk] = et
                if tk > 0:
                    ep = ets[tk - 1]
                    nc.tensor.matmul(
                        po0, v_t[b][:, tk - 1, :], ep[:, 0:TT],
                        start=(tk == 1), stop=False,
                    )
                    nc.tensor.matmul(
                        po1, v_t[b][:, tk - 1, :], ep[:, TT:2 * TT],
                        start=(tk == 1), stop=False,
                    )
                if tk == 1:
                    nc.vector.tensor_tensor(
                        out=sacc, in0=ets[0], in1=et, op=add
                    )
                elif tk > 1:
                    nc.vector.tensor_tensor(out=sacc, in0=sacc, in1=et, op=add)
                yield
            ep = ets[NK - 1]
            nc.tensor.matmul(
                po0, v_t[b][:, NK - 1, :], ep[:, 0:TT], start=False, stop=True
            )
            nc.tensor.matmul(
                po1, v_t[b][:, NK - 1, :], ep[:, TT:2 * TT], start=False, stop=True
            )
            # release the po PSUM banks immediately (the denominator chain
            # must not gate the next superjob's pv matmuls)
            posb = attp.tile([128, 2, TT], f16, tag="posb")
            nc.vector.tensor_copy(out=posb[:, 0, :], in_=po0)
            nc.vector.tensor_copy(out=posb[:, 1, :], in_=po1)
            # pre-epilogue boundary: the scheduler resumes us a few units
            # into the NEXT superjob so the pd/pb matmuls below don't
            # head-of-line-block its score stream on the PE queue while the
            # sacc chain drains on the vector engine.
            yield
            # denominator: partition-sum via ones-matmul, recip, broadcast
            pd = pstream.tile([1, 2 * TT], f32, tag="ps", name="pd")
            nc.tensor.matmul(pd[:, 0:TT], ones, sacc[:, 0:TT],
                             start=True, stop=True)
            nc.tensor.matmul(pd[:, TT:2 * TT], ones, sacc[:, TT:2 * TT],
                             start=True, stop=True)
            if DEBUG and hc == 0 and b == 0 and tq0 == 0:
                nc.sync.dma_start(out=dbg_sacc.ap(), in_=sacc)
            rdf = statp.tile([1, 2 * TT], f32, tag="stat")
            nc.vector.reciprocal_approx_fast(out=rdf, in_=pd)
            rd = statp.tile([1, 2 * TT], f16, tag="stat")
            nc.vector.tensor_copy(out=rd, in_=rdf)
            pb = pstream.tile([128, 2 * TT], f32, tag="ps", name="pba")
            nc.tensor.matmul(pb[:, 0:TT], ones_k1, rd[:, 0:TT],
                             start=True, stop=True)
            nc.tensor.matmul(pb[:, TT:2 * TT], ones_k1, rd[:, TT:2 * TT],
                             start=True, stop=True)
            aout = attp.tile([128, 2, TT], f16, tag="aout")
            nc.vector.tensor_tensor(
                out=aout[:, 0, :], in0=posb[:, 0, :], in1=pb[:, 0:TT], op=mult
            )
            nc.vector.tensor_tensor(
                out=aout[:, 1, :], in0=posb[:, 1, :], in1=pb[:, TT:2 * TT],
                op=mult,
            )
            j0 = b * NTT + tq0
            nc.gpsimd.dma_start(
                out=cc_in[hc].ap()[j0:j0 + 2, :, :].rearrange("j p t -> p j t"),
                in_=aout,
            )

        ag_t = [None, None]

        def do_a2a(hc):
            nc.gpsimd.collective_compute(
                "AllToAll",
                mybir.AluOpType.bypass,
                replica_groups=[list(range(NCORES))],
                ins=[cc_in[hc].ap()],
                outs=[cc_out[hc].ap()],
            )
            aga = qkvp.tile([128, 4, TPC], f16, tag=f"ag{hc}", bufs=2,
                            name=f"ag{hc}a")
            agb = qkvp.tile([128, 4, TPC], f16, tag=f"ag{hc}", bufs=2,
                            name=f"ag{hc}b")
            for j in range(NCORES):
                dst = aga if j < 4 else agb
                nc.gpsimd.dma_start(
                    out=dst[:, j % 4, :],
                    in_=cc_out[hc].ap()[j, :, :],
                )
            ag_t[hc] = (aga, agb)

        # wo weight streams (contiguous thanks to host relayout)
        wos_e = {}
        wos_o = {}

        def load_wos(dc, parity):
            store = wos_e if parity == 0 else wos_o
            pool = wop if parity == 0 else wop2
            store[dc] = pool.tile(
                [128, 8, 128], f16, tag=f"wo{parity}", name=f"wos{parity}_{dc}"
            )
            nc.sync.dma_start(out=store[dc], in_=woR.ap()[dc, parity, :, :, :])

        ye_t = {}

        def even_pass(dc):
            aga, agb = ag_t[0]
            py = pacc.tile([128, TPC], f32, tag="pacc", name="pye")
            for j in range(NCORES):
                srct = aga if j < 4 else agb
                nc.tensor.matmul(
                    py, wos_e[dc][:, j, :], srct[:, j % 4, :],
                    start=(j == 0), stop=(j == 7),
                )
            ye = yep.tile([128, TPC], f16, tag="ye", name=f"ye{dc}")
            nc.vector.tensor_copy(out=ye, in_=py)
            ye_t[dc] = ye

        def odd_pass(dc):
            aga, agb = ag_t[1]
            py = pacc.tile([128, TPC], f32, tag="pacc", name="pyo")
            for j in range(NCORES):
                srct = aga if j < 4 else agb
                nc.tensor.matmul(
                    py, wos_o[dc][:, j, :], srct[:, j % 4, :],
                    start=(j == 0), stop=(j == 7),
                )
            yt = yp.tile([128, TPC], f32, tag="y")
            nc.vector.tensor_tensor(out=yt, in0=py, in1=ye_t[dc], op=add)
            nc.gpsimd.dma_start(out=yT[dc * 128:(dc + 1) * 128, :], in_=yt)

        # ---- schedule ----
        # Phase A: first two proj tiles dense (nothing else is ready).
        for _ in gen_proj(0, 0, nsplit=8):
            pass
        for _ in gen_proj(0, 1):
            pass

        # Interleaved phases: attention superjobs with the remaining proj
        # stream woven in (ratio keeps kh/q producers ahead of consumers).
        proj_rest = chain(
            gen_proj(0, 2), gen_proj(0, 3),
            gen_proj(1, 0), gen_proj(1, 1), gen_proj(1, 2), gen_proj(1, 3),
        )
        proj_alive = True
        proj_done = 0
        att_done = 0
        att_jobs = [
            (0, 0, 0), (0, 0, 2), (0, 1, 0), (0, 1, 2),
            (1, 0, 0), (1, 0, 2), (1, 1, 0), (1, 1, 2),
        ]

        def pull_proj(n):
            nonlocal proj_alive, proj_done
            for _ in range(n):
                if not proj_alive:
                    return
                try:
                    next(proj_rest)
                    proj_done += 1
                except StopIteration:
                    proj_alive = False
                    # the last tile's deferred v transposes must be emitted
                    # before any attention unit that consumes them
                    flush_transposes()

        def finish(gen):
            if gen is not None:
                for _ in gen:
                    pass

        # prime the PE queue with a few proj units so the first superjob's
        # stt/kh producers have latency cover
        pull_proj(4)

        # wo weight loads ride the sync queue tail (emitted after every x
        # load, so they never park ahead of anything latency-critical)
        wo_load_sched = [(dc, 0) for dc in range(NDC)] + [
            (dc, 1) for dc in range(8)
        ]
        wo_li = 0
        pend = None            # previous superjob, paused pre-epilogue
        pend_fire_a2a = False  # emit A2A#1 right after it finishes
        for jidx, (hc, b, tq0) in enumerate(att_jobs):
            g = gen_att(hc, b, tq0)
            units = 0
            for _ in g:
                units += 1
                if units > NK:
                    break  # pre-epilogue boundary reached
                att_done += 1
                if units == 6 and pend is not None:
                    finish(pend)
                    pend = None
                    if pend_fire_a2a:
                        do_a2a(0)
                        pend_fire_a2a = False
                while proj_alive and proj_done < 6 + att_done * 1.4:
                    pull_proj(1)
                if jidx >= 4 and wo_li < len(wo_load_sched) and att_done % 2 == 0:
                    dc, parity = wo_load_sched[wo_li]
                    load_wos(dc, parity)
                    wo_li += 1
            pend = g
            if jidx == 3:
                pend_fire_a2a = True
        finish(pend)
        while proj_alive:
            pull_proj(1)
        flush_transposes()
        while wo_li < len(wo_load_sched):
            dc, parity = wo_load_sched[wo_li]
            load_wos(dc, parity)
            wo_li += 1

        do_a2a(1)

        if DEBUG:
            for h in range(HPC):
                for b in range(B):
                    nc.sync.dma_start(
                        out=dbg_q.ap()[:, h, b * L:(b + 1) * L], in_=qh_t[h][b]
                    )
            for b in range(B):
                nc.sync.dma_start(
                    out=dbg_k.ap()[:, b * L:(b + 1) * L], in_=kh_t[b]
                )
                nc.sync.dma_start(
                    out=dbg_v.ap()[:, b * 16:(b + 1) * 16, :], in_=v_t[b]
                )
            for j in range(NCORES):
                a0 = ag_t[0][0] if j < 4 else ag_t[0][1]
                a1 = ag_t[1][0] if j < 4 else ag_t[1][1]
                nc.sync.dma_start(out=dbg_ag.ap()[:, 2 * j, :], in_=a0[:, j % 4, :])
                nc.sync.dma_start(out=dbg_ag.ap()[:, 2 * j + 1, :], in_=a1[:, j % 4, :])

        # even half of the out-projection overlaps the second AllToAll
        for dc in range(NDC):
            even_pass(dc)
        # tail wos_o loads interleave with the odd passes: a slot is only
        # re-allocated after its previous tenant's reader is emitted (the
        # pool WAR tracking can't see future readers)
        for dc in range(NDC):
            odd_pass(dc)
            if dc + 8 < NDC:
                load_wos(dc + 8, 1)

    nc.finalize()
    return nc


def kernel(x, wq, wk, wv, wo, qn_w, kn_w):
    from concourse.bass_utils import run_bass_kernel_spmd

    if "nc" not in _CACHE:
        _CACHE["nc"] = _build_nc()
    nc = _CACHE["nc"]

    x = np.asarray(x, dtype=np.float32)
    wq = np.asarray(wq, dtype=np.float32)
    wk = np.asarray(wk, dtype=np.float32)
    wv = np.asarray(wv, dtype=np.float32)
    wo = np.asarray(wo, dtype=np.float32)
    qn_w = np.asarray(qn_w, dtype=np.float32).reshape(HD, 1).copy()
    kn_w = np.asarray(kn_w, dtype=np.float32).reshape(HD, 1).copy()

    xT = x.reshape(T, D).T.astype(np.float16)          # [D, T]
    xT6 = np.ascontiguousarray(
        xT.reshape(NDC, 128, B * NTT, TT).transpose(2, 1, 0, 3)
    )                                                   # [j, p, dc, t]
    wo16 = wo.astype(np.float16)
    woR = np.ascontiguousarray(
        wo16.reshape(8, 2, 128, NDC, 128).transpose(3, 1, 2, 0, 4)
    )                                                   # [dc, parity, p, j, m]
    cos, sin = _rope_tables()
    cosF = np.ascontiguousarray(
        np.concatenate([cos, cos], axis=0).astype(np.float16)
    )                                                   # [128, L]
    sinW = np.ascontiguousarray(
        np.concatenate([-sin, sin], axis=0).astype(np.float16)
    )                                                   # [128, L]

    in_maps = []
    for c in range(NCORES):
        wqkv_c = np.ascontiguousarray(
            np.concatenate(
                [
                    wq[:, c * HPC * HD:(c + 1) * HPC * HD],
                    wk[:, c * HD:(c + 1) * HD],
                    wv[:, c * HD:(c + 1) * HD],
                ],
                axis=1,
            ).astype(np.float16).reshape(NDC, 128, 512)
        )
        in_maps.append(
            {
                "xT6": xT6,
                "wqkv": wqkv_c,
                "woR": woR,
                "lcosF": cosF,
                "lsinW": sinW,
                "qn": qn_w,
                "kn": kn_w,
            }
        )

    trace = bool(_CACHE.get("trace"))
    r = run_bass_kernel_spmd(
        nc, in_maps, core_ids=list(range(NCORES)), trace=trace
    )
    _CACHE["last_result"] = r

    y = np.empty((T, D), dtype=np.float32)
    for c in range(NCORES):
        y[c * TPC:(c + 1) * TPC, :] = r.results[c]["yT"].T
    return y.reshape(B, L, D)


# revision 57
# speedup vs baseline: 1.0669x; 1.0669x over previous
"""Trainium2 Bass kernel for GQA attention (B=2, L=2048, D=2048, H=16, KV=8, HD=128).

Sharding: tensor-parallel over heads across 8 cores (2 Q heads + 1 KV head per
core), flash-style attention per core, then two AllToAlls (one per local head)
to redistribute from head-sharding to token-sharding before the output
projection (each core computes 512 full output rows; host concatenates).

v2 restructure vs baseline:
- exp runs at N=1024 over a 2-bank PSUM score tile (two query tiles of the
  same head share one ACT op) halving ACT per-op overhead.
- V transpose via the DMA xbar (dma_start_transpose) instead of PE.
- QKV projection and attention are software-pipelined at chunk granularity
  (attention on batch b starts as soon as the proj tiles it needs exist).
- Startup DMAs split across queues (w on scalar, x on sync, LUT on gpsimd)
  with per-chunk weight loads so the first matmul issues ~2us in.
- wo is host-relayout so every weight DMA is fully contiguous.
"""
import math
import numpy as np

B, L, D = 2, 2048, 2048
H, KV, HD = 16, 8, 128
NCORES = 8
T = B * L            # 4096 tokens, b-major
TPC = T // NCORES    # 512 tokens per core after A2A
HPC = H // NCORES    # 2 local query heads
EPS = 1e-5
ROPE_BASE = 10000.0
SCALE = HD ** -0.5

TT = 512             # token tile (free dim)
NTT = L // TT        # 4 token tiles per batch
NDC = D // 128       # 16 contraction chunks
NFC = 4              # output col chunks of 128 in qkv proj (2 q heads + k + v)
NK = L // 128        # 16 key chunks per batch

_CACHE = {}
DEBUG = False


def _rope_tables():
    """cos/sin LUTs [64, L] computed exactly like the jax reference (f32, cpu)."""
    import jax
    import jax.numpy as jnp

    cpu = jax.devices("cpu")[0]
    with jax.default_device(cpu):
        base = ROPE_BASE * 1.0 ** (HD / (HD - 2))
        freqs = base ** (jnp.arange(0, HD, 2, dtype=jnp.float32) / HD)   # [64]
        pos = jnp.arange(L, dtype=jnp.float32)                           # [L]
        angles = pos[:, None] * freqs[None, :]                           # [L, 64]
        cos = np.asarray(jnp.cos(angles), dtype=np.float32).T.copy()     # [64, L]
        sin = np.asarray(jnp.sin(angles), dtype=np.float32).T.copy()
    return cos, sin


def _build_nc():
    import concourse.bass as bass
    import concourse.tile as tile
    import concourse.mybir as mybir
    import concourse.bass_isa as bass_isa
    from concourse import bacc
    from contextlib import ExitStack
    from itertools import chain

    f32 = mybir.dt.float32
    f16 = mybir.dt.float16
    Exp = mybir.ActivationFunctionType.Exp
    Ln = mybir.ActivationFunctionType.Ln
    mult = mybir.AluOpType.mult
    add = mybir.AluOpType.add
    sub = mybir.AluOpType.subtract

    from concourse import bacc as _bacc_mod

    if not getattr(_bacc_mod, "_act_table_patch", False):
        _orig_get = _bacc_mod.get_activation_tables

        def _patched_get(arch):
            t = _orig_get(arch)
            exp = mybir.ActivationFunctionType.Exp
            ln = mybir.ActivationFunctionType.Ln
            for name, funcs in t.items():
                if name != "natural_log_exp_and_others":
                    funcs.discard(exp)
                    funcs.discard(ln)
            return t

        _bacc_mod.get_activation_tables = _patched_get
        _bacc_mod._act_table_patch = True

    nc = bacc.Bacc(num_devices=NCORES)

    # host-relayout inputs (all DMAs contiguous)
    xT6 = nc.dram_tensor("xT6", [B * NTT, 128, NDC, TT], f16, kind="ExternalInput")
    wqkv = nc.dram_tensor("wqkv", [NDC, 128, 512], f16, kind="ExternalInput")
    woR = nc.dram_tensor("woR", [NDC, 2, 128, 8, 128], f16, kind="ExternalInput")
    # full-height rope LUTs: lcosF = [cos; cos], lsinW = [-sin; sin] so
    # rope is 3 full-width DVE ops (t_a + t_b with a partition-swapped rsrc)
    lcosF = nc.dram_tensor("lcosF", [128, L], f16, kind="ExternalInput")
    lsinW = nc.dram_tensor("lsinW", [128, L], f16, kind="ExternalInput")
    qn = nc.dram_tensor("qn", [HD, 1], f32, kind="ExternalInput")
    kn = nc.dram_tensor("kn", [HD, 1], f32, kind="ExternalInput")
    yT = nc.dram_tensor("yT", [D, TPC], f32, kind="ExternalOutput")

    cc_in = [nc.dram_tensor(f"cc_in{h}", [NCORES, HD, TPC], f16) for h in range(HPC)]
    cc_out = [nc.dram_tensor(f"cc_out{h}", [NCORES, HD, TPC], f16) for h in range(HPC)]

    if DEBUG:
        dbg_q = nc.dram_tensor("dbg_q", [128, HPC, T], f16, kind="ExternalOutput")
        dbg_k = nc.dram_tensor("dbg_k", [128, T], f16, kind="ExternalOutput")
        dbg_v = nc.dram_tensor("dbg_v", [128, T // 128, HD], f16, kind="ExternalOutput")
        dbg_sacc = nc.dram_tensor("dbg_sacc", [128, 2 * TT], f16, kind="ExternalOutput")
        dbg_ag = nc.dram_tensor("dbg_ag", [128, NDC, TPC], f16, kind="ExternalOutput")

    with tile.TileContext(nc) as tc, ExitStack() as ctx, nc.allow_low_precision(
        reason="f16 storage; all matmul accumulation is fp32 PSUM"
    ):
        consts = ctx.enter_context(tc.tile_pool(name="consts", bufs=1))
        xtp = ctx.enter_context(tc.tile_pool(name="xtp", bufs=2))
        qkvp = ctx.enter_context(tc.tile_pool(name="qkvp", bufs=1))
        ropep = ctx.enter_context(tc.tile_pool(name="ropep", bufs=2))
        halfp = ctx.enter_context(tc.tile_pool(name="halfp", bufs=3))
        statp = ctx.enter_context(tc.tile_pool(name="statp", bufs=2))
        expp = ctx.enter_context(tc.tile_pool(name="expp", bufs=16))
        saccp = ctx.enter_context(tc.tile_pool(name="saccp", bufs=2))
        attp = ctx.enter_context(tc.tile_pool(name="attp", bufs=2))
        wop = ctx.enter_context(tc.tile_pool(name="wop", bufs=8))
        wop2 = ctx.enter_context(tc.tile_pool(name="wop2", bufs=8))
        yp = ctx.enter_context(tc.tile_pool(name="yp", bufs=2))
        yep = ctx.enter_context(tc.tile_pool(name="yep", bufs=16))

        pacc = ctx.enter_context(tc.tile_pool(name="pacc", bufs=2, space="PSUM"))
        pop = ctx.enter_context(tc.tile_pool(name="pop", bufs=2, space="PSUM"))
        pstream = ctx.enter_context(tc.tile_pool(name="pstream", bufs=2, space="PSUM"))

        # ---- constants ----
        # (rope LUT loads are emitted mid-schedule, behind the first x tiles,
        # to keep the startup path clear)
        eps_t = consts.tile([1, 1], f32)
        nc.vector.memset(eps_t, EPS)
        ones_f = consts.tile([128, 1], f32)
        nc.vector.memset(ones_f, 1.0)
        ones = consts.tile([128, 1], f16)
        nc.vector.tensor_copy(out=ones, in_=ones_f)
        ones_k1_f = consts.tile([1, 128], f32)
        nc.vector.memset(ones_k1_f, 1.0)
        ones_k1 = consts.tile([1, 128], f16)
        nc.vector.tensor_copy(out=ones_k1, in_=ones_k1_f)
        cos_sb = consts.tile([128, L], f16)
        sin_sb = consts.tile([128, L], f16)
        qn_sb = consts.tile([HD, 1], f32)
        kn_sb = consts.tile([HD, 1], f32)

        def load_luts():
            nc.scalar.dma_start(out=cos_sb, in_=lcosF[:, :])
            nc.scalar.dma_start(out=sin_sb, in_=lsinW[:, :])
            nc.scalar.dma_start(out=qn_sb, in_=qn[:, :])
            nc.scalar.dma_start(out=kn_sb, in_=kn[:, :])

        # ---- persistent activations ----
        qh_t = [
            [
                qkvp.tile([128, L], f16, tag=f"ag{h}", bufs=2, name=f"qh{h}{b}")
                for b in range(B)
            ]
            for h in range(HPC)
        ]
        kh_t = [
            qkvp.tile([128, L], f16, tag=f"kh{b}", name=f"kh{b}") for b in range(B)
        ]
        v_t = [
            qkvp.tile([128, L // 128, HD], f16, tag=f"v{b}", name=f"v{b}")
            for b in range(B)
        ]

        # ---- qkv weights: per-dc chunk loads on the scalar queue ----
        w_sb = consts.tile([128, NDC, 512], f16)
        for dc in range(4):
            nc.scalar.dma_start(out=w_sb[:, dc, :], in_=wqkv.ap()[dc, :, :])
        load_luts()
        for dc in range(4, NDC):
            nc.scalar.dma_start(out=w_sb[:, dc, :], in_=wqkv.ap()[dc, :, :])

        deferred_tr = []

        def flush_transposes():
            for fn in deferred_tr:
                fn()
            deferred_tr.clear()

        def gen_proj(b, tt, nsplit=1):
            """QKV projection + RoPE + RMSNorm for one 512-token tile.

            Yields once per 8 emitted matmuls so attention units can
            interleave on the PE queue. The whole x tile arrives as ONE
            contiguous 2MB DMA (host relayout); the first tile splits it
            so the first matmul isn't gated on the full transfer."""
            pos0 = tt * TT
            j = b * NTT + tt
            flush_transposes()  # previous tile's v transposes (data ready)
            xt2 = xtp.tile([128, NDC, TT], f16, tag="xt")
            step = NDC // nsplit
            for s in range(nsplit):
                nc.sync.dma_start(
                    out=xt2[:, s * step:(s + 1) * step, :],
                    in_=xT6.ap()[j, :, s * step:(s + 1) * step, :],
                )
            for fc in range(NFC):
                pp = pacc.tile([128, TT], f32, tag="pacc", name=f"pp{fc}")
                for dc in range(NDC):
                    nc.tensor.matmul(
                        pp,
                        w_sb[:, dc, fc * 128:(fc + 1) * 128],
                        xt2[:, dc, :],
                        start=(dc == 0),
                        stop=(dc == NDC - 1),
                    )
                    if dc == 7:
                        yield
                if fc < 3:
                    # rope as 3 full-width DVE ops:
                    #   roped = rsrc*[cos;cos] + [x2;x1]*[-sin;sin]
                    # where [x2;x1] is a DMA partition-swap of rsrc.
                    rsrc = ropep.tile([128, TT], f16, tag="rsrc")
                    nc.vector.tensor_copy(out=rsrc, in_=pp)
                    rsw = ropep.tile([128, TT], f16, tag="rsw")
                    nc.sync.dma_start(out=rsw[0:64, :], in_=rsrc[64:128, :])
                    nc.sync.dma_start(out=rsw[64:128, :], in_=rsrc[0:64, :])
                    ta = halfp.tile([128, TT], f16, tag="half")
                    tb = halfp.tile([128, TT], f16, tag="half")
                    roped = ropep.tile([128, TT], f16, tag="roped")
                    nc.vector.tensor_tensor(
                        out=ta, in0=rsrc, in1=cos_sb[:, pos0:pos0 + TT], op=mult
                    )
                    nc.vector.tensor_tensor(
                        out=tb, in0=rsw, in1=sin_sb[:, pos0:pos0 + TT], op=mult
                    )
                    nc.vector.tensor_tensor(out=roped, in0=ta, in1=tb, op=add)
                    sq = ropep.tile([128, TT], f16, tag="sq")
                    nc.vector.tensor_tensor(out=sq, in0=roped, in1=roped, op=mult)
                    # sum of squares over HD (partition) via ones-matmul
                    pss = pstream.tile([1, TT], f32, tag="ps", name="pss")
                    nc.tensor.matmul(pss, ones, sq, start=True, stop=True)
                    # rstd = exp(-0.5*ln(ss/HD + eps)); Ln/Exp share one table
                    lnt = statp.tile([1, TT], f32, tag="stat")
                    nc.scalar.activation(
                        out=lnt, in_=pss, func=Ln, bias=eps_t, scale=1.0 / HD
                    )
                    srd = statp.tile([1, TT], f16, tag="stat")
                    nc.scalar.activation(out=srd, in_=lnt, func=Exp, scale=-0.5)
                    # broadcast rstd over partitions via K=1 matmul
                    pb = pstream.tile([128, TT], f32, tag="ps", name="pbp")
                    nc.tensor.matmul(pb, ones_k1, srd, start=True, stop=True)
                    w_head = qn_sb if fc < 2 else kn_sb
                    if fc < 2:
                        dst = qh_t[fc][b][:, pos0:pos0 + TT]
                    else:
                        dst = kh_t[b][:, pos0:pos0 + TT]
                    nc.vector.scalar_tensor_tensor(
                        out=dst, in0=roped, scalar=w_head, in1=pb,
                        op0=mult, op1=mult,
                    )
                else:
                    # v: copy out; DMA-xbar transposes are deferred so they
                    # never park the sync queue waiting on this cast
                    vt = ropep.tile([128, TT], f16, tag="vtt", bufs=2)
                    nc.vector.tensor_copy(out=vt, in_=pp)

                    def mk(vt=vt, b=b, tt=tt):
                        for i in range(TT // 128):
                            nc.sync.dma_start_transpose(
                                out=v_t[b][:, tt * 4 + i, :],
                                in_=vt[:, i * 128:(i + 1) * 128],
                            )

                    deferred_tr.append(mk)

        def gen_att(hc, b, tq0):
            """Attention superjob: one head, two query tiles (tq0, tq0+1).

            Scores for both query tiles of key-chunk tk land in one 2-bank
            PSUM tile -> one N=1024 exp. pv matmuls trail by one chunk so
            the PE never waits on the ACT engine. Softmax denominator via
            PE ones-matmuls (latency-free next to the score stream). Yields
            once per key chunk."""
            tq1 = tq0 + 1
            qs0 = qh_t[hc][b][:, tq0 * TT:(tq0 + 1) * TT]
            qs1 = qh_t[hc][b][:, tq1 * TT:(tq1 + 1) * TT]
            po0 = pop.tile([128, TT], f32, tag="po", name=f"po{hc}{b}{tq0}a")
            po1 = pop.tile([128, TT], f32, tag="po", name=f"po{hc}{b}{tq0}b")
            sacc = saccp.tile([128, 2 * TT], f16, tag="sacc")
            ets = [None] * NK
            for tk in range(NK):
                ps = pstream.tile([128, 2 * TT], f32, tag="ps")
                kc = kh_t[b][:, tk * 128:(tk + 1) * 128]
                nc.tensor.matmul(ps[:, 0:TT], kc, qs0, start=True, stop=True)
                nc.tensor.matmul(ps[:, TT:2 * TT], kc, qs1, start=True, stop=True)
                et = expp.tile([128, 2 * TT], f16, tag="et")
                nc.scalar.activation(out=et, in_=ps, func=Exp, scale=SCALE)
                ets[tk] = et
                if tk > 0:
                    ep = ets[tk - 1]
                    nc.tensor.matmul(
                        po0, v_t[b][:, tk - 1, :], ep[:, 0:TT],
                        start=(tk == 1), stop=False,
                    )
                    nc.tensor.matmul(
                        po1, v_t[b][:, tk - 1, :], ep[:, TT:2 * TT],
                        start=(tk == 1), stop=False,
                    )
                yield
            ep = ets[NK - 1]
            nc.tensor.matmul(
                po0, v_t[b][:, NK - 1, :], ep[:, 0:TT], start=False, stop=True
            )
            nc.tensor.matmul(
                po1, v_t[b][:, NK - 1, :], ep[:, TT:2 * TT], start=False, stop=True
            )
            # dense sacc reduction: all et tiles stay resident so the adds
            # run back-to-back at superjob end instead of ticking at unit
            # cadence on the FIFO vector queue (which lags everything behind)
            nc.vector.tensor_tensor(out=sacc, in0=ets[0], in1=ets[1], op=add)
            for tk in range(2, NK):
                nc.vector.tensor_tensor(
                    out=sacc, in0=sacc, in1=ets[tk], op=add
                )
            # release the po PSUM banks immediately (the denominator chain
            # must not gate the next superjob's pv matmuls)
            posb = attp.tile([128, 2, TT], f16, tag="posb")
            nc.vector.tensor_copy(out=posb[:, 0, :], in_=po0)
            nc.vector.tensor_copy(out=posb[:, 1, :], in_=po1)
            # pre-epilogue boundary: the scheduler resumes us a few units
            # into the NEXT superjob so the pd/pb matmuls below don't
            # head-of-line-block its score stream on the PE queue while the
            # sacc chain drains on the vector engine.
            yield
            # denominator: partition-sum via ones-matmul, recip, broadcast
            pd = pstream.tile([1, 2 * TT], f32, tag="ps", name="pd")
            nc.tensor.matmul(pd[:, 0:TT], ones, sacc[:, 0:TT],
                             start=True, stop=True)
            nc.tensor.matmul(pd[:, TT:2 * TT], ones, sacc[:, TT:2 * TT],
                             start=True, stop=True)
            if DEBUG and hc == 0 and b == 0 and tq0 == 0:
                nc.sync.dma_start(out=dbg_sacc.ap(), in_=sacc)
            rdf = statp.tile([1, 2 * TT], f32, tag="stat")
            nc.vector.reciprocal_approx_fast(out=rdf, in_=pd)
            rd = statp.tile([1, 2 * TT], f16, tag="stat")
            nc.vector.tensor_copy(out=rd, in_=rdf)
            pb = pstream.tile([128, 2 * TT], f32, tag="ps", name="pba")
            nc.tensor.matmul(pb[:, 0:TT], ones_k1, rd[:, 0:TT],
                             start=True, stop=True)
            nc.tensor.matmul(pb[:, TT:2 * TT], ones_k1, rd[:, TT:2 * TT],
                             start=True, stop=True)
            aout = attp.tile([128, 2, TT], f16, tag="aout")
            nc.vector.tensor_tensor(
                out=aout[:, 0, :], in0=posb[:, 0, :], in1=pb[:, 0:TT], op=mult
            )
            nc.vector.tensor_tensor(
                out=aout[:, 1, :], in0=posb[:, 1, :], in1=pb[:, TT:2 * TT],
                op=mult,
            )
            j0 = b * NTT + tq0
            nc.gpsimd.dma_start(
                out=cc_in[hc].ap()[j0:j0 + 2, :, :].rearrange("j p t -> p j t"),
                in_=aout,
            )

        ag_t = [None, None]

        def do_a2a(hc):
            nc.gpsimd.collective_compute(
                "AllToAll",
                mybir.AluOpType.bypass,
                replica_groups=[list(range(NCORES))],
                ins=[cc_in[hc].ap()],
                outs=[cc_out[hc].ap()],
            )
            aga = qkvp.tile([128, 4, TPC], f16, tag=f"ag{hc}", bufs=2,
                            name=f"ag{hc}a")
            agb = qkvp.tile([128, 4, TPC], f16, tag=f"ag{hc}", bufs=2,
                            name=f"ag{hc}b")
            for j in range(NCORES):
                dst = aga if j < 4 else agb
                nc.gpsimd.dma_start(
                    out=dst[:, j % 4, :],
                    in_=cc_out[hc].ap()[j, :, :],
                )
            ag_t[hc] = (aga, agb)

        # wo weight streams (contiguous thanks to host relayout)
        wos_e = {}
        wos_o = {}

        def load_wos(dc, parity):
            store = wos_e if parity == 0 else wos_o
            pool = wop if parity == 0 else wop2
            store[dc] = pool.tile(
                [128, 8, 128], f16, tag=f"wo{parity}", name=f"wos{parity}_{dc}"
            )
            nc.sync.dma_start(out=store[dc], in_=woR.ap()[dc, parity, :, :, :])

        ye_t = {}

        def even_pass(dc):
            aga, agb = ag_t[0]
            py = pacc.tile([128, TPC], f32, tag="pacc", name="pye")
            for j in range(NCORES):
                srct = aga if j < 4 else agb
                nc.tensor.matmul(
                    py, wos_e[dc][:, j, :], srct[:, j % 4, :],
                    start=(j == 0), stop=(j == 7),
                )
            ye = yep.tile([128, TPC], f16, tag="ye", name=f"ye{dc}")
            nc.vector.tensor_copy(out=ye, in_=py)
            ye_t[dc] = ye

        def odd_pass(dc):
            aga, agb = ag_t[1]
            py = pacc.tile([128, TPC], f32, tag="pacc", name="pyo")
            for j in range(NCORES):
                srct = aga if j < 4 else agb
                nc.tensor.matmul(
                    py, wos_o[dc][:, j, :], srct[:, j % 4, :],
                    start=(j == 0), stop=(j == 7),
                )
            yt = yp.tile([128, TPC], f32, tag="y")
            nc.vector.tensor_tensor(out=yt, in0=py, in1=ye_t[dc], op=add)
            nc.gpsimd.dma_start(out=yT[dc * 128:(dc + 1) * 128, :], in_=yt)

        # ---- schedule ----
        # Phase A: first two proj tiles dense (nothing else is ready).
        for _ in gen_proj(0, 0, nsplit=8):
            pass
        for _ in gen_proj(0, 1):
            pass

        # Interleaved phases: attention superjobs with the remaining proj
        # stream woven in (ratio keeps kh/q producers ahead of consumers).
        proj_rest = chain(
            gen_proj(0, 2), gen_proj(0, 3),
            gen_proj(1, 0), gen_proj(1, 1), gen_proj(1, 2), gen_proj(1, 3),
        )
        proj_alive = True
        proj_done = 0
        att_done = 0
        att_jobs = [
            (0, 0, 0), (0, 0, 2), (0, 1, 0), (0, 1, 2),
            (1, 0, 0), (1, 0, 2), (1, 1, 0), (1, 1, 2),
        ]

        def pull_proj(n):
            nonlocal proj_alive, proj_done
            for _ in range(n):
                if not proj_alive:
                    return
                try:
                    next(proj_rest)
                    proj_done += 1
                except StopIteration:
                    proj_alive = False
                    # the last tile's deferred v transposes must be emitted
                    # before any attention unit that consumes them
                    flush_transposes()

        def finish(gen):
            if gen is not None:
                for _ in gen:
                    pass

        # prime the PE queue with a few proj units so the first superjob's
        # stt/kh producers have latency cover
        pull_proj(4)

        # wo weight loads ride the sync queue tail (emitted after every x
        # load, so they never park ahead of anything latency-critical)
        wo_load_sched = [(dc, 0) for dc in range(8)] + [
            (dc, 1) for dc in range(8)
        ]
        wo_li = 0
        pend = None            # previous superjob, paused pre-epilogue
        pend_fire_a2a = False  # emit A2A#1 right after it finishes
        for jidx, (hc, b, tq0) in enumerate(att_jobs):
            g = gen_att(hc, b, tq0)
            units = 0
            for _ in g:
                units += 1
                if units > NK:
                    break  # pre-epilogue boundary reached
                att_done += 1
                if units == 6 and pend is not None:
                    finish(pend)
                    pend = None
                    if pend_fire_a2a:
                        do_a2a(0)
                        pend_fire_a2a = False
                while proj_alive and proj_done < 6 + att_done * 1.4:
                    pull_proj(1)
                if jidx >= 4 and wo_li < len(wo_load_sched) and att_done % 2 == 0:
                    dc, parity = wo_load_sched[wo_li]
                    load_wos(dc, parity)
                    wo_li += 1
            pend = g
            if jidx == 3:
                pend_fire_a2a = True
        finish(pend)
        while proj_alive:
            pull_proj(1)
        flush_transposes()
        while wo_li < len(wo_load_sched):
            dc, parity = wo_load_sched[wo_li]
            load_wos(dc, parity)
            wo_li += 1

        do_a2a(1)

        if DEBUG:
            for h in range(HPC):
                for b in range(B):
                    nc.sync.dma_start(
                        out=dbg_q.ap()[:, h, b * L:(b + 1) * L], in_=qh_t[h][b]
                    )
            for b in range(B):
                nc.sync.dma_start(
                    out=dbg_k.ap()[:, b * L:(b + 1) * L], in_=kh_t[b]
                )
                nc.sync.dma_start(
                    out=dbg_v.ap()[:, b * 16:(b + 1) * 16, :], in_=v_t[b]
                )
            for j in range(NCORES):
                a0 = ag_t[0][0] if j < 4 else ag_t[0][1]
                a1 = ag_t[1][0] if j < 4 else ag_t[1][1]
                nc.sync.dma_start(out=dbg_ag.ap()[:, 2 * j, :], in_=a0[:, j % 4, :])
                nc.sync.dma_start(out=dbg_ag.ap()[:, 2 * j + 1, :], in_=a1[:, j % 4, :])

        # even half of the out-projection overlaps the second AllToAll.
        # Tail weight loads interleave with the passes: a pool slot is only
        # re-allocated after its previous tenant's reader is emitted (the
        # pool WAR tracking can't see future readers).
        for dc in range(NDC):
            even_pass(dc)
            if dc + 8 < NDC:
                load_wos(dc + 8, 0)
        for dc in range(NDC):
            odd_pass(dc)
            if dc + 8 < NDC:
                load_wos(dc + 8, 1)

    nc.finalize()
    return nc


def kernel(x, wq, wk, wv, wo, qn_w, kn_w):
    from concourse.bass_utils import run_bass_kernel_spmd

    if "nc" not in _CACHE:
        _CACHE["nc"] = _build_nc()
    nc = _CACHE["nc"]

    x = np.asarray(x, dtype=np.float32)
    wq = np.asarray(wq, dtype=np.float32)
    wk = np.asarray(wk, dtype=np.float32)
    wv = np.asarray(wv, dtype=np.float32)
    wo = np.asarray(wo, dtype=np.float32)
    qn_w = np.asarray(qn_w, dtype=np.float32).reshape(HD, 1).copy()
    kn_w = np.asarray(kn_w, dtype=np.float32).reshape(HD, 1).copy()

    xT = x.reshape(T, D).T.astype(np.float16)          # [D, T]
    xT6 = np.ascontiguousarray(
        xT.reshape(NDC, 128, B * NTT, TT).transpose(2, 1, 0, 3)
    )                                                   # [j, p, dc, t]
    wo16 = wo.astype(np.float16)
    woR = np.ascontiguousarray(
        wo16.reshape(8, 2, 128, NDC, 128).transpose(3, 1, 2, 0, 4)
    )                                                   # [dc, parity, p, j, m]
    cos, sin = _rope_tables()
    cosF = np.ascontiguousarray(
        np.concatenate([cos, cos], axis=0).astype(np.float16)
    )                                                   # [128, L]
    sinW = np.ascontiguousarray(
        np.concatenate([-sin, sin], axis=0).astype(np.float16)
    )                                                   # [128, L]

    in_maps = []
    for c in range(NCORES):
        wqkv_c = np.ascontiguousarray(
            np.concatenate(
                [
                    wq[:, c * HPC * HD:(c + 1) * HPC * HD],
                    wk[:, c * HD:(c + 1) * HD],
                    wv[:, c * HD:(c + 1) * HD],
                ],
                axis=1,
            ).astype(np.float16).reshape(NDC, 128, 512)
        )
        in_maps.append(
            {
                "xT6": xT6,
                "wqkv": wqkv_c,
                "woR": woR,
                "lcosF": cosF,
                "lsinW": sinW,
                "qn": qn_w,
                "kn": kn_w,
            }
        )

    trace = bool(_CACHE.get("trace"))
    r = run_bass_kernel_spmd(
        nc, in_maps, core_ids=list(range(NCORES)), trace=trace
    )
    _CACHE["last_result"] = r

    y = np.empty((T, D), dtype=np.float32)
    for c in range(NCORES):
        y[c * TPC:(c + 1) * TPC, :] = r.results[c]["yT"].T
    return y.reshape(B, L, D)
